# revision 13
# baseline (speedup 1.0000x reference)
"""MoE-over-image Trainium2 kernel (nn_MoEImage).

Data-parallel over batch: 8 cores x 4 samples. Per core, samples are
processed as 2 pairs; a pair occupies the two 64-partition halves of
SBUF/PSUM so every [K=64, M=64] matmul packs 2x2 into PE quadrants.

v2 changes over the 244us baseline (evidence: CoreSim cost-model
timeline; PE was 81.9% busy / 192.8us, ACT 71.4% / 168us):
  - Input x is staged per-pair as [128, HWT] in DRAM; ONE DMA per
    chunk (both samples) instead of two, and pair 0 rides the SP
    queue while pair 1 rides the Pool queue -> Phase A is no longer
    DMA-queue-bound (was 2.9us/chunk vs 2.0us ACT).
  - Combine restructured: PE only does the routed half (2 K=128
    matmuls with the gathered per-sample weights); the shared half
    (0.5*(s0+s1), diagonal weights) moved to DVE as 2 cross-
    partition-base tensor_adds [64,512] bf16 (2x mode) + 1
    scalar_tensor_tensor (T1*0.5 + po) that also does the PSUM
    evacuation. Cuts 2 serial K=128 matmuls + 1 DVE copy per chunk.
  - s/r matmul emission order diagonal-first ((0,0),(64,64),(0,64),
    (64,0)) so LDWEIGHTS of the opposite row-group pulls ahead of the
    in-flight matmul (PE reorder window) -> 2x quadrant concurrency.
  - Gating gf assembly uses cross-partition-base DVE copies instead
    of SBUF->SBUF DMAs (saves ~3.4us of serial gating latency).
  - Output DMA merged per pair ([128,1024] per transfer).

Timing harness: MOE_KERNEL_TIME_ITERS=N wraps the body in a tc.For_i
hardware loop and reports loop-delta wall time via _timed_exec.
"""

import os
import numpy as np

B, CIN, H, W = 32, 64, 128, 128
HID, OUT, E, S, RR = 64, 64, 8, 2, 16
TOP_K = 2
TEMP = 2.0
BN_EPS = 1e-5

NCORES = 8
BPC = B // NCORES          # samples per core = 4
NPAIR = BPC // 2           # 2
HWT = H * W                # 16384
CHUNK_A = 2048
NCH_A = HWT // CHUNK_A     # 8
CHUNK_B = 512
NCH_B = HWT // CHUNK_B     # 32
OGRP = 4                   # output chunks per staging group

_CACHE = {}
LAST_RESULTS = None

# packed-constant layouts: (name, (rows, cols))
PACKF = [("feb", (128, 1)), ("sb0", (128, 1)), ("sb1", (128, 1)),
         ("ist", (128, 64)),
         ("cidx", (128, 8)), ("l1", (64, 128)), ("b1", (128, 1)),
         ("ca1", (128, 8)), ("bca1", (8, 1)), ("ca2", (8, 128)),
         ("bca2", (128, 1)), ("l2", (128, 64)), ("b2", (64, 1)),
         ("g3b", (65, 8)), ("ssel", (4, 512))]
PACKB = [("wf", (128, 64)), ("ws", (128, 128))]
FCOLS = sum(c for _, (_, c) in PACKF)
BCOLS = sum(c for _, (_, c) in PACKB)


def _build_program(loop_iters: int = 1, ebz: bool = True, szb: bool = True):
    import concourse.bass as bass
    import concourse.mybir as mybir
    from concourse import bacc
    from concourse.tile import TileContext

    f32 = mybir.dt.float32
    bf16 = mybir.dt.bfloat16
    u16 = mybir.dt.uint16
    u32 = mybir.dt.uint32
    GELU = mybir.ActivationFunctionType.Gelu
    TANH = mybir.ActivationFunctionType.Tanh

    nc = bacc.Bacc(
        "TRN2",
        target_bir_lowering=False,
        debug=False,
        enable_asserts=False,
        num_devices=NCORES,
    )

    # ---- DRAM I/O ----  (x/y staged pair-major: [pair, 128, HWT])
    x_d = nc.dram_tensor("x", [NPAIR, 128, HWT], bf16, kind="ExternalInput").ap()
    y_d = nc.dram_tensor("y", [NPAIR, 128, HWT], bf16, kind="ExternalOutput").ap()

    def inp(name, shape, dt=None):
        return nc.dram_tensor(name, shape, dt or f32, kind="ExternalInput").ap()

    packb_d = inp("packb", [128, BCOLS], bf16)   # wf | ws
    we_d = inp("we", [128, E * 64], bf16)    # expert e .T at cols e*64, dup rows
    packf_d = inp("packf", [128, FCOLS])     # all small f32 consts
    ebt_d = inp("ebt", [128, E])       # e_b[e, p%64] (gather data, separate)

    import contextlib

    with TileContext(nc) as tc:
        with (
            tc.tile_pool(name="consts", bufs=1) as cpool,
            tc.tile_pool(name="fstore", bufs=1) as fpool,
            tc.tile_pool(name="route", bufs=1) as rpool,
            tc.tile_pool(name="work", bufs=2) as wpool,
        ):
            # ---- load constants ----
            # packb (wf/ws: needed immediately) on the SP queue, ahead of x.
            pb_t = cpool.tile_from(packb_d)
            # gating/routing consts ride the Pool queue so SP starts
            # streaming pair-0 x right away.
            we = cpool.tile_from(we_d, forced_dma_engine=mybir.EngineType.Pool)
            pf_t = cpool.tile_from(packf_d,
                                   forced_dma_engine=mybir.EngineType.Pool)
            ebt = cpool.tile_from(ebt_d,
                                  forced_dma_engine=mybir.EngineType.Pool)
            wf = pb_t[:, 0:64]
            ws = pb_t[:, 64:192]
            cv = {}
            _off = 0
            for _nm, (_r, _c) in PACKF:
                cv[_nm] = pf_t[0:_r, _off:_off + _c]
                _off += _c
            feb, sb0, sb1 = cv["feb"], cv["sb0"], cv["sb1"]
            ist, cidx = cv["ist"], cv["cidx"]
            l1, b1, ca1, bca1 = cv["l1"], cv["b1"], cv["ca1"], cv["bca1"]
            ca2, bca2, l2, b2 = cv["ca2"], cv["bca2"], cv["l2"], cv["b2"]
            g3b, ssel = cv["g3b"], cv["ssel"]

            loop_cm = (tc.For_i(0, loop_iters, 1,
                                hint_engines=tuple(mybir.ALL_ENGINES))
                       if loop_iters > 1 else contextlib.nullcontext())
            with loop_cm:
                _kernel_body(nc, tc, mybir, cpool, fpool, rpool, wpool, x_d,
                             y_d, wf, ws, we, feb, sb0, sb1, ebt, ist, cidx,
                             ssel, l1, b1, ca1, bca1, ca2, bca2, l2, b2, g3b,
                             GELU, TANH, f32, bf16, u16, u32, ebz, szb)

    nc.compile()
    return nc


def _kernel_body(nc, tc, mybir, cpool, fpool, rpool, wpool, x_d, y_d, wf, ws,
                 we, feb, sb0, sb1, ebt, ist, cidx, ssel, l1, b1, ca1, bca1,
                 ca2, bca2, l2, b2, g3b, GELU, TANH, f32, bf16, u16, u32, ebz,
                 szb):
            # features store per pair: [128, HWT] bf16 (32KB/partition each)
            fstore = []
            for p in range(NPAIR):
                ft = fpool.tile([128, HWT], bf16, tag=f"F{p}", name=f"F{p}")
                fstore.append(ft)

            gfp = []  # per-pair pooled feature sums [128,1]
            # ---------------- Phase A: features + pooled sum ----------------
            with tc.tile_pool(name="psumA", bufs=2, space="PSUM") as pA:
                for p in range(NPAIR):
                    inq = nc.sync if p == 0 else nc.gpsimd
                    gfacc = rpool.tile([128, NCH_A], f32, tag=f"gfacc{p}",
                                       name=f"gfacc{p}")
                    for j in range(NCH_A):
                        ca = j * CHUNK_A
                        xt = wpool.tile([128, CHUNK_A], bf16, tag="xin",
                                        name=f"xt_{p}_{j}")
                        inq.dma_start(out=xt, in_=x_d[p][:, ca:ca + CHUNK_A])
                        pf = pA.tile([128, CHUNK_A], f32, tag="pf",
                                     name=f"pf_{p}_{j}")
                        for h in range(0, CHUNK_A, 512):
                            nc.tensor.matmul(
                                pf[0:64, h:h + 512],
                                lhsT=wf[0:64, :],
                                rhs=xt[0:64, h:h + 512],
                                start=True, stop=True)
                            nc.tensor.matmul(
                                pf[64:128, h:h + 512],
                                lhsT=wf[64:128, :],
                                rhs=xt[64:128, h:h + 512],
                                start=True, stop=True)
                        nc.scalar.activation(
                            fstore[p][:, ca:ca + CHUNK_A], pf, GELU,
                            bias=feb, accum_out=gfacc[:, j:j + 1])
                    g = rpool.tile([128, 1], f32, tag=f"gfp{p}", name=f"gfp{p}")
                    nc.vector.reduce_sum(g, gfacc, axis=mybir.AxisListType.X)
                    gfp.append(g)

            # ---------------- Gating ----------------
            with tc.tile_pool(name="psumG", bufs=1, space="PSUM") as pG:
                # gf as [64(hid), 4(sample)] via cross-partition-base copies
                gft = rpool.tile([64, 4], f32, tag="gft")
                for p in range(NPAIR):
                    nc.vector.tensor_copy(gft[:, 2 * p:2 * p + 1],
                                          gfp[p][0:64, 0:1])
                    nc.vector.tensor_copy(gft[:, 2 * p + 1:2 * p + 2],
                                          gfp[p][64:128, 0:1])

                ph1 = pG.tile([128, 4], f32, tag="ph1")
                nc.tensor.matmul(ph1, lhsT=l1, rhs=gft, start=True, stop=True)
                h1t = rpool.tile([128, 4], f32, tag="h1t")
                nc.scalar.activation(h1t, ph1, GELU, bias=b1)

                pa1 = pG.tile([8, 4], f32, tag="pa1")
                nc.tensor.matmul(pa1, lhsT=ca1, rhs=h1t, start=True, stop=True)
                a1 = rpool.tile([8, 4], f32, tag="a1")
                nc.scalar.activation(a1, pa1, GELU, bias=bca1)

                patt = pG.tile([128, 4], f32, tag="patt")
                nc.tensor.matmul(patt, lhsT=ca2, rhs=a1, start=True, stop=True)
                # sigmoid(2*att) = 0.5 + 0.5*tanh(att);  att = patt + bca2
                att_t = rpool.tile([128, 4], f32, tag="att_t")
                nc.scalar.activation(att_t, patt, TANH, bias=bca2)
                gate = rpool.tile([128, 4], f32, tag="gate")
                nc.vector.tensor_scalar(
                    gate, att_t, 0.5, scalar2=0.5,
                    op0=mybir.AluOpType.mult, op1=mybir.AluOpType.add)
                h1m = rpool.tile([128, 4], f32, tag="h1m")
                nc.vector.tensor_mul(h1m, h1t, gate)

                phh = pG.tile([64, 4], f32, tag="phh")
                nc.tensor.matmul(phh, lhsT=l2, rhs=h1m, start=True, stop=True)
                hhx = rpool.tile([65, 4], f32, tag="hhx")
                nc.vector.memset(hhx[64:65, :], 1.0)
                nc.scalar.activation(hhx[0:64, :], phh, GELU, bias=b2)

                psc = pG.tile([4, 8], f32, tag="psc")
                nc.tensor.matmul(psc, lhsT=hhx, rhs=g3b, start=True, stop=True)
                scores = rpool.tile([4, 8], f32, tag="scores")
                nc.vector.tensor_copy(scores, psc)

                vals = rpool.tile([4, 8], f32, tag="vals")
                idxs = rpool.tile([4, 8], u32, tag="idxs")
                nc.vector.max_with_indices(vals, idxs, scores)

                dv = rpool.tile([4, 1], f32, tag="dv")
                nc.vector.tensor_sub(dv, vals[:, 0:1], vals[:, 1:2])
                th = rpool.tile([4, 1], f32, tag="th")
                nc.scalar.activation(th, dv, TANH, scale=1.0 / (2.0 * TEMP))
                # u columns: [i1, i2, w1, w2]
                u = rpool.tile([4, 4], f32, tag="u")
                nc.vector.tensor_copy(u[:, 0:1], idxs[:, 0:1])
                nc.vector.tensor_copy(u[:, 1:2], idxs[:, 1:2])
                nc.vector.tensor_scalar(
                    u[:, 2:3], th, 0.5, scalar2=0.5,
                    op0=mybir.AluOpType.mult, op1=mybir.AluOpType.add)
                nc.vector.tensor_scalar(
                    u[:, 3:4], u[:, 2:3], -1.0, scalar2=1.0,
                    op0=mybir.AluOpType.mult, op1=mybir.AluOpType.add)

                # per-sample routing data
                wsel = []
                crw = []
                ebias = []
                for b in range(BPC):
                    pbc = pG.tile([128, 4], f32, tag="pbc")
                    nc.tensor.matmul(
                        pbc, lhsT=ssel[:, b * 128:(b + 1) * 128], rhs=u,
                        start=True, stop=True)
                    bc = rpool.tile([128, 4], f32, tag=f"bc{b}", name=f"bc{b}")
                    nc.vector.tensor_copy(bc, pbc)

                    # combine weights lhsT: [w1*I; w2*I]
                    wm = rpool.tile([128, 1], f32, tag=f"wm{b}", name=f"wm{b}")
                    nc.vector.tensor_copy(wm[0:64, :], bc[0:64, 2:3])
                    nc.vector.tensor_copy(wm[64:128, :], bc[64:128, 3:4])
                    cr = rpool.tile([128, 64], bf16, tag=f"cr{b}", name=f"cr{b}")
                    nc.vector.tensor_mul(cr, ist, wm.to_broadcast((128, 64)))
                    crw.append(cr)

                    # gather indices for expert weight columns
                    idxf = rpool.tile([128, 8], f32, tag="idxf")
                    s1 = rpool.tile([128, 1], f32, tag="s1c")
                    s2 = rpool.tile([128, 1], f32, tag="s2c")
                    nc.vector.tensor_scalar_mul(s1, bc[:, 0:1], 64.0)
                    nc.vector.tensor_scalar_mul(s2, bc[:, 1:2], 64.0)
                    nc.vector.tensor_add(idxf[:, 0:4], cidx[:, 0:4],
                                         s1.to_broadcast((128, 4)))
                    nc.vector.tensor_add(idxf[:, 4:8], cidx[:, 4:8],
                                         s2.to_broadcast((128, 4)))
                    idxu = rpool.tile([128, 8], u16, tag=f"idxu{b}",
                                      name=f"idxu{b}")
                    nc.vector.tensor_copy(idxu, idxf)
                    wsb = rpool.tile([128, 128], bf16, tag=f"wsel{b}",
                                     name=f"wsel{b}")
                    nc.gpsimd.indirect_copy(wsb, data=we, idxs=idxu,
                                            i_know_ap_gather_is_preferred=True)
                    wsel.append(wsb)

                    # expert bias gather: [e_b[i1]; e_b[i2]]
                    ebf = rpool.tile([128, 1], f32, tag="ebf")
                    nc.vector.tensor_copy(ebf[0:64, :], bc[0:64, 0:1])
                    nc.vector.tensor_copy(ebf[64:128, :], bc[64:128, 1:2])
                    ebu = rpool.tile([128, 1], u16, tag=f"ebu{b}",
                                     name=f"ebu{b}")
                    nc.vector.tensor_copy(ebu, ebf)
                    ebb = rpool.tile([128, 1], f32, tag=f"ebias{b}",
                                     name=f"ebias{b}")
                    nc.gpsimd.indirect_copy(ebb, data=ebt, idxs=ebu,
                                            i_know_ap_gather_is_preferred=True)
                    ebias.append(ebb)

            # ---------------- Phase B ----------------
            # chunk = 512 output cols; psum: pS (2 banks x2 bufs), pR
            # (2 banks), pO (1 bank x2). PE: shared/routed quadrant
            # matmuls + 2 routed-combine K=128 matmuls. DVE: shared sum
            # (2x bf16) + fused (0.5*T1 + po) -> ost. Combine is one
            # chunk behind so PE never blocks ACT.
            with (
                tc.tile_pool(name="psumS", bufs=2, space="PSUM") as pS,
                tc.tile_pool(name="psumR", bufs=1, space="PSUM") as pR,
                tc.tile_pool(name="psumO", bufs=2, space="PSUM") as pO,
            ):
                for p in range(NPAIR):
                    F = fstore[p]
                    cr0, cr1 = crw[2 * p], crw[2 * p + 1]
                    ws0, ws1 = wsel[2 * p], wsel[2 * p + 1]
                    eb0, eb1 = ebias[2 * p], ebias[2 * p + 1]
                    pend = None  # (gs, gr, i) awaiting combine
                    st = {"ost": None}
                    for i in range(NCH_B):
                        cc = i * CHUNK_B
                        rt = F[0:64, cc:cc + CHUNK_B]
                        rb = F[64:128, cc:cc + CHUNK_B]
                        # shared experts, sample-major partitions:
                        # ps[0:64] = sample a (cols 0:512 s0, 512:1024 s1),
                        # ps[64:128] = sample b. The s0+s1 sum is then a
                        # SAME-base-partition column add (walrus requires
                        # equal bases for SBUF+SBUF TensorTensor).
                        ps = pS.tile([128, 2 * CHUNK_B], f32, tag="s",
                                     name=f"ps_{p}_{i}")
                        nc.tensor.matmul(ps[0:64, 0:CHUNK_B],
                                         lhsT=ws[0:64, 0:64], rhs=rt,
                                         start=True, stop=True)
                        nc.tensor.matmul(ps[64:128, CHUNK_B:],
                                         lhsT=ws[64:128, 64:128], rhs=rb,
                                         start=True, stop=True)
                        nc.tensor.matmul(ps[0:64, CHUNK_B:],
                                         lhsT=ws[0:64, 64:128], rhs=rt,
                                         start=True, stop=True)
                        nc.tensor.matmul(ps[64:128, 0:CHUNK_B],
                                         lhsT=ws[64:128, 0:64], rhs=rb,
                                         start=True, stop=True)
                        gs = wpool.tile([128, 2 * CHUNK_B], bf16, tag="gs",
                                        name=f"gs_{p}_{i}")
                        if szb:
                            nc.scalar.activation(gs, ps, GELU)
                        else:
                            nc.scalar.activation(gs[:, 0:CHUNK_B],
                                                 ps[:, 0:CHUNK_B], GELU,
                                                 bias=sb0)
                            nc.scalar.activation(gs[:, CHUNK_B:],
                                                 ps[:, CHUNK_B:], GELU,
                                                 bias=sb1)

                        # routed experts
                        pr = pR.tile([128, 2 * CHUNK_B], f32, tag="r",
                                     name=f"pr_{p}_{i}")
                        nc.tensor.matmul(pr[0:64, 0:CHUNK_B],
                                         lhsT=ws0[0:64, 0:64], rhs=rt,
                                         start=True, stop=True)
                        nc.tensor.matmul(pr[64:128, CHUNK_B:],
                                         lhsT=ws1[64:128, 64:128], rhs=rb,
                                         start=True, stop=True)
                        nc.tensor.matmul(pr[64:128, 0:CHUNK_B],
                                         lhsT=ws0[0:64, 64:128], rhs=rt,
                                         start=True, stop=True)
                        nc.tensor.matmul(pr[0:64, CHUNK_B:],
                                         lhsT=ws1[64:128, 0:64], rhs=rb,
                                         start=True, stop=True)
                        gr = wpool.tile([128, 2 * CHUNK_B], bf16, tag="gr",
                                        name=f"gr_{p}_{i}")
                        if ebz:
                            nc.scalar.activation(gr, pr, GELU)
                        else:
                            nc.scalar.activation(gr[:, 0:CHUNK_B],
                                                 pr[:, 0:CHUNK_B], GELU,
                                                 bias=eb0)
                            nc.scalar.activation(gr[:, CHUNK_B:],
                                                 pr[:, CHUNK_B:], GELU,
                                                 bias=eb1)

                        if pend is not None:
                            _emit_combine(nc, mybir, pO, wpool, y_d, cr0, cr1,
                                          p, *pend, f32, bf16, st)
                        pend = (gs, gr, i)
                    _emit_combine(nc, mybir, pO, wpool, y_d, cr0, cr1, p,
                                  *pend, f32, bf16, st)


def _emit_combine(nc, mybir, pO, wpool, y_d, cr0, cr1, p, gs, gr, i, f32,
                  bf16, st):
    C = CHUNK_B
    cc = i * C
    # routed halves on PE (dynamic per-sample weights)
    po = pO.tile([128, C], f32, tag="po", name=f"po_{p}_{i}")
    nc.tensor.matmul(po[0:64, :], lhsT=cr0, rhs=gr[:, 0:C],
                     start=True, stop=True)
    nc.tensor.matmul(po[64:128, :], lhsT=cr1, rhs=gr[:, C:],
                     start=True, stop=True)
    # shared halves on DVE: same-base column adds (s0 + s1 per sample)
    t1 = wpool.tile([128, C], bf16, tag="t1", name=f"t1_{p}_{i}")
    nc.vector.tensor_add(t1, gs[:, 0:C], gs[:, C:])
    k = i % OGRP
    if k == 0:
        st["ost"] = wpool.tile([128, OGRP * C], bf16, tag="ost",
                               name=f"ost_{p}_{i}")
    ost = st["ost"]
    # ost_slice = 0.5*T1 + po   (also evacuates the PSUM bank)
    nc.vector.scalar_tensor_tensor(
        ost[:, k * C:(k + 1) * C], in0=t1, scalar=0.5, in1=po,
        op0=mybir.AluOpType.mult, op1=mybir.AluOpType.add)
    if k % 2 == 1:
        HG = 2 * C
        hb = (k // 2) * HG
        cg = (i - k) * C + hb
        nc.gpsimd.dma_start(out=y_d[p][:, cg:cg + HG],
                            in_=ost[:, hb:hb + HG])


def _host_consts(fe_w, fe_b, s_w, s_b, e_w, e_b, g1_w, g1_b, bn1_g, bn1_b,
                 ca1_w, ca1_b, ca2_w, ca2_b, g2_w, g2_b, bn2_g, bn2_b,
                 g3_w, g3_b):
    f = np.float32
    I64 = np.eye(64, dtype=f)

    def dup(a):  # duplicate along partition dim
        return np.concatenate([a, a], axis=0).astype(f)

    wf = dup(fe_w.T)                                   # [128, 64]
    ws = dup(np.concatenate([s_w[0].T, s_w[1].T], axis=1))   # [128,128]
    we = dup(np.concatenate([e_w[e].T for e in range(E)], axis=1))  # [128,512]
    feb = np.concatenate([fe_b, fe_b]).reshape(128, 1).astype(f)
    sb0 = np.concatenate([s_b[0], s_b[0]]).reshape(128, 1).astype(f)
    sb1 = np.concatenate([s_b[1], s_b[1]]).reshape(128, 1).astype(f)
    ebt = np.concatenate([e_b.T, e_b.T], axis=0).astype(f)   # [128, E]
    ist = np.concatenate([I64, I64], axis=0)

    pm = np.arange(128) % 16
    jj = np.arange(8)
    cidx = (pm[:, None] + 16 * jj[None, :] - 64 * (jj[None, :] >= 4)).astype(f)

    ssel = np.zeros((4, 4 * 128), dtype=f)
    for b in range(4):
        ssel[b, b * 128:(b + 1) * 128] = 1.0

    s1 = (bn1_g / np.sqrt(1.0 + BN_EPS)).astype(f)
    l1 = ((g1_w * s1[:, None]) / float(HWT)).T.astype(f)     # [64, 128]
    b1 = (g1_b * s1 + bn1_b).reshape(128, 1).astype(f)
    ca1 = ca1_w.T.astype(f)                                  # [128, 8]
    bca1 = ca1_b.reshape(8, 1).astype(f)
    ca2 = ca2_w.T.astype(f)                                  # [8, 128]
    bca2 = ca2_b.reshape(128, 1).astype(f)
    s2 = (bn2_g / np.sqrt(1.0 + BN_EPS)).astype(f)
    l2 = (g2_w * s2[:, None]).T.astype(f)                    # [128, 64]
    b2 = (g2_b * s2 + bn2_b).reshape(64, 1).astype(f)
    g3b = np.concatenate([g3_w.T, g3_b.reshape(1, 8)], axis=0).astype(f)

    return dict(wf=wf, ws=ws, we=we, feb=feb, sb0=sb0, sb1=sb1, ebt=ebt,
                ist=ist, cidx=cidx, ssel=ssel, l1=l1, b1=b1, ca1=ca1,
                bca1=bca1, ca2=ca2, bca2=bca2, l2=l2, b2=b2, g3b=g3b)


def _timed_exec(nc, in_maps, n_cores, ncalls=24):
    """Run nc's NEFF on all cores via PJRT with pre-uploaded inputs and no
    host materialization; return min wall seconds per call."""
    import time
    import jax
    from jax.sharding import Mesh, PartitionSpec, NamedSharding
    from jax.experimental.shard_map import shard_map
    import concourse.mybir as mybir
    from concourse.bass2jax import (
        _bass_exec_p, install_neuronx_cc_hook, partition_id_tensor)

    install_neuronx_cc_hook()
    partition_name = (nc.partition_id_tensor.name
                      if nc.partition_id_tensor else None)
    in_names, out_names, out_avals, zero_outs = [], [], [], []
    for alloc in nc.m.functions[0].allocations:
        if not isinstance(alloc, mybir.MemoryLocationSet):
            continue
        name = alloc.memorylocations[0].name
        if alloc.kind == "ExternalInput":
            if name != partition_name:
                in_names.append(name)
        elif alloc.kind == "ExternalOutput":
            out_names.append(name)
            shape = tuple(alloc.tensor_shape)
            dtype = mybir.dt.np(alloc.dtype)
            out_avals.append(jax.core.ShapedArray(shape, dtype))
            zero_outs.append(np.zeros(shape, dtype))
    n_params = len(in_names)
    all_in = in_names + out_names + ([partition_name] if partition_name else [])

    def _body(*args):
        operands = list(args)
        if partition_name is not None:
            operands.append(partition_id_tensor())
        return tuple(_bass_exec_p.bind(
            *operands,
            out_avals=tuple(out_avals),
            in_names=tuple(all_in),
            out_names=tuple(out_names),
            lowering_input_output_aliases=(),
            sim_require_finite=True,
            sim_require_nnan=True,
            nc=nc,
        ))

    devices = jax.devices()[:n_cores]
    mesh = Mesh(np.array(devices), ("core",))
    nin = n_params + len(out_names)
    sharded = jax.jit(
        shard_map(_body, mesh=mesh, in_specs=(PartitionSpec("core"),) * nin,
                  out_specs=(PartitionSpec("core"),) * len(out_names),
                  check_rep=False),
        keep_unused=True)
    concat = [np.concatenate([np.asarray(in_maps[c][nm])
                              for c in range(n_cores)], axis=0)
              for nm in in_names]
    concat += [np.zeros((n_cores * z.shape[0], *z.shape[1:]), z.dtype)
               for z in zero_outs]
    sh = NamedSharding(mesh, PartitionSpec("core"))
    dev_in = [jax.device_put(a, sh) for a in concat]
    outs = sharded(*dev_in)
    jax.block_until_ready(outs)  # compile + warm
    best = None
    for _ in range(ncalls):
        t0 = time.perf_counter()
        outs = sharded(*dev_in)
        jax.block_until_ready(outs)
        dt = time.perf_counter() - t0
        best = dt if best is None else min(best, dt)
    return best


def kernel(**inputs):
    global LAST_RESULTS
    import sys
    if "/opt/trn_rl_repo" not in sys.path:
        sys.path.insert(0, "/opt/trn_rl_repo")
    from concourse import bass_utils

    import ml_dtypes
    bf = ml_dtypes.bfloat16
    x = np.ascontiguousarray(np.asarray(inputs["x"], dtype=np.float32).astype(bf))
    consts = _host_consts(**{k: np.asarray(v, np.float32)
                             for k, v in inputs.items() if k != "x"})
    packb = np.zeros((128, BCOLS), dtype=bf)
    off = 0
    for nm, (r, c) in PACKB:
        packb[0:r, off:off + c] = consts[nm].astype(bf)
        off += c
    packf = np.zeros((128, FCOLS), dtype=np.float32)
    off = 0
    for nm, (r, c) in PACKF:
        packf[0:r, off:off + c] = consts[nm]
        off += c
    consts = {"packb": packb, "packf": packf,
              "we": consts["we"].astype(bf), "ebt": consts["ebt"]}

    ebz = bool(np.all(np.asarray(inputs["e_b"]) == 0.0))
    szb = bool(np.all(np.asarray(inputs["s_b"]) == 0.0))
    key = ("nc", ebz, szb)
    if key not in _CACHE:
        _CACHE[key] = _build_program(1, ebz, szb)
    nc = _CACHE[key]

    # stage x pair-major: per core [NPAIR, 128, HWT]
    xr = x.reshape(B, CIN, HWT)
    in_maps = []
    for c in range(NCORES):
        xc = xr[c * BPC:(c + 1) * BPC]                     # [4, 64, HWT]
        xp = np.ascontiguousarray(
            xc.reshape(NPAIR, 2 * CIN, HWT))               # [2, 128, HWT]
        m = {"x": xp}
        m.update(consts)
        in_maps.append(m)

    res = bass_utils.run_bass_kernel_spmd(
        nc, in_maps, core_ids=list(range(NCORES)), trace=False)
    out = np.concatenate(
        [np.asarray(r["y"], dtype=np.float32).reshape(BPC, OUT, HWT)
         for r in res.results], axis=0)

    iters = int(os.environ.get("MOE_KERNEL_TIME_ITERS", "0"))
    ref_iters = int(os.environ.get("MOE_KERNEL_TIME_REF", "1"))
    exec_ns = None
    if iters > 1:
        keyk = ("nc", ebz, szb, iters)
        if keyk not in _CACHE:
            _CACHE[keyk] = _build_program(iters, ebz, szb)
        if ref_iters > 1:
            keyr = ("nc", ebz, szb, ref_iters)
            if keyr not in _CACHE:
                _CACHE[keyr] = _build_program(ref_iters, ebz, szb)
            t1 = _timed_exec(_CACHE[keyr], in_maps, NCORES)
        else:
            t1 = _timed_exec(nc, in_maps, NCORES)
        tk = _timed_exec(_CACHE[keyk], in_maps, NCORES)
        exec_ns = (tk - t1) / (iters - ref_iters) * 1e9
        print(f"[timing] wall {ref_iters}-iter {t1*1e3:.2f} ms, {iters}-iter "
              f"{tk*1e3:.2f} ms -> per-iter {exec_ns:.0f} ns")

    import types
    LAST_RESULTS = types.SimpleNamespace(
        results=res.results, exec_time_ns=exec_ns,
        mean_exec_time_ns=None, max_exec_time_core_id=None)
    return out.reshape(B, OUT, H, W)


# revision 27
# speedup vs baseline: 1.1979x; 1.1979x over previous
"""MoE-over-image Trainium2 kernel (nn_MoEImage).

Data-parallel over batch: 8 cores x 4 samples. Per core, samples are
processed as 2 pairs; a pair occupies the two 64-partition halves of
SBUF/PSUM so every [K=64, M=64] matmul packs 2x2 into PE quadrants
(concurrent on real HW). The kernel is ScalarE-bound: ~21M gelu
elements/core at 1 elem/cycle/lane (plus ~350 cycles/ACTIVATE fixed
overhead) is a ~180us floor; everything else is overlapped under it.

Pipeline per core:
  x/y are staged pair-major in DRAM ([pair, 128, HWT]) so every input
  chunk is ONE [128, 2048] DMA; pair 0 streams on the SP queue, pair 1
  on the Pool queue (two DMA queues in parallel). Gating/routing
  consts (we/packf/ebt) also ride Pool so SP starts on x immediately.
  Phase A: features = gelu(fe_w @ x) streamed in chunks; the gating
           pooled-sum is accumulated per-chunk on the otherwise-idle
           DVE (reduce over the just-written fstore slice).
  Gating:  tiny MLP on [feat, sample]-oriented tiles, top-2 via
           vector.max_with_indices, softmax-of-2 via tanh (tanh shares
           the gelu ACT table set -> no table reloads), per-sample
           expert weights gathered with gpsimd.indirect_copy. gf is
           assembled with cross-base DVE copies (no SBUF->SBUF DMA).
  Phase B: per 512-col chunk: shared-expert matmuls (4 quads), gelu,
           routed-expert matmuls with gathered weights, gelu, then a
           K=128 "combine" matmul folds 0.5*(s0+s1) + w1*e1 + w2*e2
           across partition halves (cs loaded once: cs,cs,cr0,cr1
           order with per-partition-range accumulation groups).
           PSUM: pS 1 buf, pR 2 bufs (routed matmuls run ahead),
           pO 2 bufs. DVE copies PSUM->SBUF into a 4-chunk staging
           buffer; one [128, 2048] output DMA per group on Pool.

Timing harness: MOE_KERNEL_TIME_ITERS=N wraps the body in a tc.For_i
hardware loop and reports (wall_N - wall_1)/(N-1) via pre-uploaded,
donation-free PJRT execution (see _timed_exec).

HW-measured ladder (two-loop, this container): prev-best 225.4us ->
infra (pair-major merged DMAs, dual queues, merged out-DMA, DVE gf
copies) 218.4 -> +DVE pooled-sum +pR double-buffer 216.2 -> +cs-LDW
dedup +2048-col out-DMA batches 215.2us/iter, rel err 4.1e-3.
Known-slower-on-HW variants: merged M=128 matmuls (+20us), scalar-queue
DMAs (+33us), 4096-col input chunks (+34us), sample-major shared PSUM
layout w/ DVE shared-sum (+38us: loses 4-quadrant packing; walrus also
forbids cross-base SBUF+SBUF TensorTensor), merged [128,2048]-ACTIVATE
with po aliased into the consumed SR bank (+110us: PE FIFO serializes
on the PE->DVE->PE bank-reuse chain). See memory notes.
"""

import os
import numpy as np

B, CIN, H, W = 32, 64, 128, 128
HID, OUT, E, S, RR = 64, 64, 8, 2, 16
TOP_K = 2
TEMP = 2.0
BN_EPS = 1e-5

NCORES = 8
BPC = B // NCORES          # samples per core = 4
NPAIR = BPC // 2           # 2
HWT = H * W                # 16384
CHUNK_A = 2048
NCH_A = HWT // CHUNK_A     # 8
CHUNK_B = 512
NCH_B = HWT // CHUNK_B     # 32
OGRP = 4                   # output chunks per staging group

_CACHE = {}
LAST_RESULTS = None

# packed-constant layouts: (name, (rows, cols))
PACKF = [("feb", (128, 1)), ("sbias", (128, 1)), ("ist", (128, 64)),
         ("cidx", (128, 8)), ("l1", (64, 128)), ("b1", (128, 1)),
         ("ca1", (128, 8)), ("bca1", (8, 1)), ("ca2", (8, 128)),
         ("bca2", (128, 1)), ("l2", (128, 64)), ("b2", (64, 1)),
         ("g3b", (65, 8)), ("ssel", (4, 512))]
PACKB = [("wf", (128, 64)), ("ws", (128, 128)), ("cs", (128, 64))]
FCOLS = sum(c for _, (_, c) in PACKF)
BCOLS = sum(c for _, (_, c) in PACKB)


def _build_program(loop_iters: int = 1, ebz: bool = True):
    import concourse.bass as bass
    import concourse.mybir as mybir
    from concourse import bacc
    from concourse.tile import TileContext

    f32 = mybir.dt.float32
    bf16 = mybir.dt.bfloat16
    u16 = mybir.dt.uint16
    u32 = mybir.dt.uint32
    GELU = mybir.ActivationFunctionType.Gelu
    TANH = mybir.ActivationFunctionType.Tanh

    nc = bacc.Bacc(
        "TRN2",
        target_bir_lowering=False,
        debug=False,
        enable_asserts=False,
        num_devices=NCORES,
    )

    # ---- DRAM I/O ----  (x/y staged pair-major: [pair, 128, HWT])
    x_d = nc.dram_tensor("x", [NPAIR, 128, HWT], bf16, kind="ExternalInput").ap()
    y_d = nc.dram_tensor("y", [NPAIR, 128, HWT], bf16, kind="ExternalOutput").ap()

    def inp(name, shape, dt=None):
        return nc.dram_tensor(name, shape, dt or f32, kind="ExternalInput").ap()

    packb_d = inp("packb", [128, BCOLS], bf16)   # wf | ws | cs
    we_d = inp("we", [128, E * 64], bf16)    # expert e .T at cols e*64, dup rows
    packf_d = inp("packf", [128, FCOLS])     # all small f32 consts
    ebt_d = inp("ebt", [128, E])       # e_b[e, p%64] (gather data, separate)

    import contextlib

    with TileContext(nc) as tc:
        with (
            tc.tile_pool(name="consts", bufs=1) as cpool,
            tc.tile_pool(name="fstore", bufs=1) as fpool,
            tc.tile_pool(name="route", bufs=1) as rpool,
            tc.tile_pool(name="work", bufs=2) as wpool,
        ):
            # ---- load constants ----
            pb_t = cpool.tile_from(packb_d)
            we = cpool.tile_from(we_d, forced_dma_engine=mybir.EngineType.Pool)
            pf_t = cpool.tile_from(packf_d,
                                   forced_dma_engine=mybir.EngineType.Pool)
            ebt = cpool.tile_from(ebt_d,
                                  forced_dma_engine=mybir.EngineType.Pool)
            wf = pb_t[:, 0:64]
            ws = pb_t[:, 64:192]
            cs = pb_t[:, 192:256]
            cv = {}
            _off = 0
            for _nm, (_r, _c) in PACKF:
                cv[_nm] = pf_t[0:_r, _off:_off + _c]
                _off += _c
            feb, sbias, ist, cidx = cv["feb"], cv["sbias"], cv["ist"], cv["cidx"]
            l1, b1, ca1, bca1 = cv["l1"], cv["b1"], cv["ca1"], cv["bca1"]
            ca2, bca2, l2, b2 = cv["ca2"], cv["bca2"], cv["l2"], cv["b2"]
            g3b, ssel = cv["g3b"], cv["ssel"]

            loop_cm = (tc.For_i(0, loop_iters, 1,
                                hint_engines=tuple(mybir.ALL_ENGINES))
                       if loop_iters > 1 else contextlib.nullcontext())
            with loop_cm:
                _kernel_body(nc, tc, mybir, cpool, fpool, rpool, wpool, x_d,
                             y_d, wf, ws, we, feb, sbias, ebt, cs, ist, cidx,
                             ssel, l1, b1, ca1, bca1, ca2, bca2, l2, b2, g3b,
                             GELU, TANH, f32, bf16, u16, u32, ebz)

    nc.compile()
    return nc


def _kernel_body(nc, tc, mybir, cpool, fpool, rpool, wpool, x_d, y_d, wf, ws,
                 we, feb, sbias, ebt, cs, ist, cidx, ssel, l1, b1, ca1, bca1,
                 ca2, bca2, l2, b2, g3b, GELU, TANH, f32, bf16, u16, u32, ebz):
            # features store per pair: [128, HWT] fp32 (64KB/partition each)
            fstore = []
            for p in range(NPAIR):
                ft = fpool.tile([128, HWT], bf16, tag=f"F{p}", name=f"F{p}")
                fstore.append(ft)

            gfp = []  # per-pair pooled feature sums [128,1]
            # ---------------- Phase A: features + pooled sum ----------------
            with tc.tile_pool(name="psumA", bufs=2, space="PSUM") as pA:
                for p in range(NPAIR):
                    inq = nc.sync if p == 0 else nc.gpsimd
                    gfacc = rpool.tile([128, NCH_A], f32, tag=f"gfacc{p}",
                                       name=f"gfacc{p}")
                    for j in range(NCH_A):
                        ca = j * CHUNK_A
                        xt = wpool.tile([128, CHUNK_A], bf16, tag="xin",
                                        name=f"xt_{p}_{j}")
                        inq.dma_start(out=xt, in_=x_d[p][:, ca:ca + CHUNK_A])
                        pf = pA.tile([128, CHUNK_A], f32, tag="pf",
                                     name=f"pf_{p}_{j}")
                        for h in range(0, CHUNK_A, 512):
                            nc.tensor.matmul(
                                pf[0:64, h:h + 512],
                                lhsT=wf[0:64, :],
                                rhs=xt[0:64, h:h + 512],
                                start=True, stop=True)
                            nc.tensor.matmul(
                                pf[64:128, h:h + 512],
                                lhsT=wf[64:128, :],
                                rhs=xt[64:128, h:h + 512],
                                start=True, stop=True)
                        nc.scalar.activation(
                            fstore[p][:, ca:ca + CHUNK_A], pf, GELU, bias=feb)
                        # pooled sum on the idle DVE (keeps ACT free of the
                        # accum_out read-out penalty)
                        nc.vector.reduce_sum(
                            gfacc[:, j:j + 1],
                            fstore[p][:, ca:ca + CHUNK_A],
                            axis=mybir.AxisListType.X)
                    g = rpool.tile([128, 1], f32, tag=f"gfp{p}", name=f"gfp{p}")
                    nc.vector.reduce_sum(g, gfacc, axis=mybir.AxisListType.X)
                    gfp.append(g)

            # ---------------- Gating ----------------
            with tc.tile_pool(name="psumG", bufs=1, space="PSUM") as pG:
                # gf as [64(hid), 4(sample)]
                gft = rpool.tile([64, 4], f32, tag="gft")
                for p in range(NPAIR):
                    nc.vector.tensor_copy(gft[:, 2 * p:2 * p + 1],
                                          gfp[p][0:64, 0:1])
                    nc.vector.tensor_copy(gft[:, 2 * p + 1:2 * p + 2],
                                          gfp[p][64:128, 0:1])

                ph1 = pG.tile([128, 4], f32, tag="ph1")
                nc.tensor.matmul(ph1, lhsT=l1, rhs=gft, start=True, stop=True)
                h1t = rpool.tile([128, 4], f32, tag="h1t")
                nc.scalar.activation(h1t, ph1, GELU, bias=b1)

                pa1 = pG.tile([8, 4], f32, tag="pa1")
                nc.tensor.matmul(pa1, lhsT=ca1, rhs=h1t, start=True, stop=True)
                a1 = rpool.tile([8, 4], f32, tag="a1")
                nc.scalar.activation(a1, pa1, GELU, bias=bca1)

                patt = pG.tile([128, 4], f32, tag="patt")
                nc.tensor.matmul(patt, lhsT=ca2, rhs=a1, start=True, stop=True)
                # sigmoid(2*att) = 0.5 + 0.5*tanh(att);  att = patt + bca2
                att_t = rpool.tile([128, 4], f32, tag="att_t")
                nc.scalar.activation(att_t, patt, TANH, bias=bca2)
                gate = rpool.tile([128, 4], f32, tag="gate")
                nc.vector.tensor_scalar(
                    gate, att_t, 0.5, scalar2=0.5,
                    op0=mybir.AluOpType.mult, op1=mybir.AluOpType.add)
                h1m = rpool.tile([128, 4], f32, tag="h1m")
                nc.vector.tensor_mul(h1m, h1t, gate)

                phh = pG.tile([64, 4], f32, tag="phh")
                nc.tensor.matmul(phh, lhsT=l2, rhs=h1m, start=True, stop=True)
                hhx = rpool.tile([65, 4], f32, tag="hhx")
                nc.vector.memset(hhx[64:65, :], 1.0)
                nc.scalar.activation(hhx[0:64, :], phh, GELU, bias=b2)

                psc = pG.tile([4, 8], f32, tag="psc")
                nc.tensor.matmul(psc, lhsT=hhx, rhs=g3b, start=True, stop=True)
                scores = rpool.tile([4, 8], f32, tag="scores")
                nc.vector.tensor_copy(scores, psc)

                vals = rpool.tile([4, 8], f32, tag="vals")
                idxs = rpool.tile([4, 8], u32, tag="idxs")
                nc.vector.max_with_indices(vals, idxs, scores)

                dv = rpool.tile([4, 1], f32, tag="dv")
                nc.vector.tensor_sub(dv, vals[:, 0:1], vals[:, 1:2])
                th = rpool.tile([4, 1], f32, tag="th")
                nc.scalar.activation(th, dv, TANH, scale=1.0 / (2.0 * TEMP))
                # u columns: [i1, i2, w1, w2]
                u = rpool.tile([4, 4], f32, tag="u")
                nc.vector.tensor_copy(u[:, 0:1], idxs[:, 0:1])
                nc.vector.tensor_copy(u[:, 1:2], idxs[:, 1:2])
                nc.vector.tensor_scalar(
                    u[:, 2:3], th, 0.5, scalar2=0.5,
                    op0=mybir.AluOpType.mult, op1=mybir.AluOpType.add)
                nc.vector.tensor_scalar(
                    u[:, 3:4], u[:, 2:3], -1.0, scalar2=1.0,
                    op0=mybir.AluOpType.mult, op1=mybir.AluOpType.add)

                # per-sample routing data
                wsel = []
                crw = []
                ebias = []
                for b in range(BPC):
                    pbc = pG.tile([128, 4], f32, tag="pbc")
                    nc.tensor.matmul(
                        pbc, lhsT=ssel[:, b * 128:(b + 1) * 128], rhs=u,
                        start=True, stop=True)
                    bc = rpool.tile([128, 4], f32, tag=f"bc{b}", name=f"bc{b}")
                    nc.vector.tensor_copy(bc, pbc)

                    # combine weights lhsT: [w1*I; w2*I]
                    wm = rpool.tile([128, 1], f32, tag=f"wm{b}", name=f"wm{b}")
                    nc.vector.tensor_copy(wm[0:64, :], bc[0:64, 2:3])
                    nc.vector.tensor_copy(wm[64:128, :], bc[64:128, 3:4])
                    cr = rpool.tile([128, 64], bf16, tag=f"cr{b}", name=f"cr{b}")
                    nc.vector.tensor_mul(cr, ist, wm.to_broadcast((128, 64)))
                    crw.append(cr)

                    # gather indices for expert weight columns
                    idxf = rpool.tile([128, 8], f32, tag="idxf")
                    s1 = rpool.tile([128, 1], f32, tag="s1c")
                    s2 = rpool.tile([128, 1], f32, tag="s2c")
                    nc.vector.tensor_scalar_mul(s1, bc[:, 0:1], 64.0)
                    nc.vector.tensor_scalar_mul(s2, bc[:, 1:2], 64.0)
                    nc.vector.tensor_add(idxf[:, 0:4], cidx[:, 0:4],
                                         s1.to_broadcast((128, 4)))
                    nc.vector.tensor_add(idxf[:, 4:8], cidx[:, 4:8],
                                         s2.to_broadcast((128, 4)))
                    idxu = rpool.tile([128, 8], u16, tag=f"idxu{b}",
                                      name=f"idxu{b}")
                    nc.vector.tensor_copy(idxu, idxf)
                    wsb = rpool.tile([128, 128], bf16, tag=f"wsel{b}",
                                     name=f"wsel{b}")
                    nc.gpsimd.indirect_copy(wsb, data=we, idxs=idxu,
                                            i_know_ap_gather_is_preferred=True)
                    wsel.append(wsb)

                    # expert bias gather: [e_b[i1]; e_b[i2]]
                    ebf = rpool.tile([128, 1], f32, tag="ebf")
                    nc.vector.tensor_copy(ebf[0:64, :], bc[0:64, 0:1])
                    nc.vector.tensor_copy(ebf[64:128, :], bc[64:128, 1:2])
                    ebu = rpool.tile([128, 1], u16, tag=f"ebu{b}",
                                     name=f"ebu{b}")
                    nc.vector.tensor_copy(ebu, ebf)
                    ebb = rpool.tile([128, 1], f32, tag=f"ebias{b}",
                                     name=f"ebias{b}")
                    nc.gpsimd.indirect_copy(ebb, data=ebt, idxs=ebu,
                                            i_know_ap_gather_is_preferred=True)
                    ebias.append(ebb)

            # ---------------- Phase B ----------------
            # chunk = 512 output cols; psum: S-tiles (2 banks x2 bufs),
            # R-tile (2 banks), O-tiles (1 bank x2). Combine is software-
            # pipelined one chunk behind so PE never blocks ACT.
            # pS single-buffered (s(i+1) refills during ACT's gr(i) window);
            # pR double-buffered so routed matmuls run chunks ahead and
            # gr(i) never waits on PE.
            with (
                tc.tile_pool(name="psumS", bufs=1, space="PSUM") as pS,
                tc.tile_pool(name="psumR", bufs=2, space="PSUM") as pR,
                tc.tile_pool(name="psumO", bufs=2, space="PSUM") as pO,
            ):
                for p in range(NPAIR):
                    F = fstore[p]
                    cr0, cr1 = crw[2 * p], crw[2 * p + 1]
                    ws0, ws1 = wsel[2 * p], wsel[2 * p + 1]
                    eb0, eb1 = ebias[2 * p], ebias[2 * p + 1]
                    pend = None  # (gs, gr, i) awaiting combine
                    st = {"ost": None}
                    for i in range(NCH_B):
                        cc = i * CHUNK_B
                        rt = F[0:64, cc:cc + CHUNK_B]
                        rb = F[64:128, cc:cc + CHUNK_B]
                        # shared experts: cols [0:512]=b0, [512:1024]=b1
                        ps = pS.tile([128, 2 * CHUNK_B], f32, tag="s",
                                     name=f"ps_{p}_{i}")
                        nc.tensor.matmul(ps[0:64, 0:CHUNK_B],
                                         lhsT=ws[0:64, 0:64], rhs=rt,
                                         start=True, stop=True)
                        nc.tensor.matmul(ps[64:128, 0:CHUNK_B],
                                         lhsT=ws[0:64, 64:128], rhs=rt,
                                         start=True, stop=True)
                        nc.tensor.matmul(ps[0:64, CHUNK_B:],
                                         lhsT=ws[64:128, 0:64], rhs=rb,
                                         start=True, stop=True)
                        nc.tensor.matmul(ps[64:128, CHUNK_B:],
                                         lhsT=ws[64:128, 64:128], rhs=rb,
                                         start=True, stop=True)
                        gs = wpool.tile([128, 2 * CHUNK_B], bf16, tag="gs",
                                        name=f"gs_{p}_{i}")
                        nc.scalar.activation(gs, ps, GELU, bias=sbias)

                        # routed experts
                        pr = pR.tile([128, 2 * CHUNK_B], f32, tag="r",
                                     name=f"pr_{p}_{i}")
                        nc.tensor.matmul(pr[0:64, 0:CHUNK_B],
                                         lhsT=ws0[0:64, 0:64], rhs=rt,
                                         start=True, stop=True)
                        nc.tensor.matmul(pr[64:128, 0:CHUNK_B],
                                         lhsT=ws0[0:64, 64:128], rhs=rt,
                                         start=True, stop=True)
                        nc.tensor.matmul(pr[0:64, CHUNK_B:],
                                         lhsT=ws1[64:128, 0:64], rhs=rb,
                                         start=True, stop=True)
                        nc.tensor.matmul(pr[64:128, CHUNK_B:],
                                         lhsT=ws1[64:128, 64:128], rhs=rb,
                                         start=True, stop=True)
                        gr = wpool.tile([128, 2 * CHUNK_B], bf16, tag="gr",
                                        name=f"gr_{p}_{i}")
                        if ebz:
                            nc.scalar.activation(gr, pr, GELU)
                        else:
                            nc.scalar.activation(gr[:, 0:CHUNK_B],
                                                 pr[:, 0:CHUNK_B], GELU,
                                                 bias=eb0)
                            nc.scalar.activation(gr[:, CHUNK_B:],
                                                 pr[:, CHUNK_B:], GELU,
                                                 bias=eb1)

                        if pend is not None:
                            _emit_combine(nc, pO, wpool, y_d, cs, cr0, cr1,
                                          p, *pend, f32, bf16, st)
                        pend = (gs, gr, i)
                    _emit_combine(nc, pO, wpool, y_d, cs, cr0, cr1, p, *pend,
                                  f32, bf16, st)


def _emit_combine(nc, pO, wpool, y_d, cs, cr0, cr1, p, gs, gr, i, f32, bf16, st):
    cc = i * CHUNK_B
    po = pO.tile([128, CHUNK_B], f32, tag="po", name=f"po_{p}_{i}")
    nc.tensor.matmul(po[0:64, :], lhsT=cs, rhs=gs[:, 0:CHUNK_B],
                     start=True, stop=False, skip_group_check=True)
    nc.tensor.matmul(po[64:128, :], lhsT=cs, rhs=gs[:, CHUNK_B:],
                     start=True, stop=False, skip_group_check=True)
    nc.tensor.matmul(po[0:64, :], lhsT=cr0, rhs=gr[:, 0:CHUNK_B],
                     start=False, stop=True, skip_group_check=True)
    nc.tensor.matmul(po[64:128, :], lhsT=cr1, rhs=gr[:, CHUNK_B:],
                     start=False, stop=True, skip_group_check=True)
    k = i % OGRP
    if k == 0:
        st["ost"] = wpool.tile([128, OGRP * CHUNK_B], bf16, tag="ost",
                               name=f"ost_{p}_{i}")
    ost = st["ost"]
    nc.vector.tensor_copy(ost[:, k * CHUNK_B:(k + 1) * CHUNK_B], po)
    if k == OGRP - 1:
        HG = OGRP * CHUNK_B
        cg = (i - k) * CHUNK_B
        nc.gpsimd.dma_start(out=y_d[p][:, cg:cg + HG], in_=ost)


def _host_consts(fe_w, fe_b, s_w, s_b, e_w, e_b, g1_w, g1_b, bn1_g, bn1_b,
                 ca1_w, ca1_b, ca2_w, ca2_b, g2_w, g2_b, bn2_g, bn2_b,
                 g3_w, g3_b):
    f = np.float32
    I64 = np.eye(64, dtype=f)

    def dup(a):  # duplicate along partition dim
        return np.concatenate([a, a], axis=0).astype(f)

    wf = dup(fe_w.T)                                   # [128, 64]
    ws = dup(np.concatenate([s_w[0].T, s_w[1].T], axis=1))   # [128,128]
    we = dup(np.concatenate([e_w[e].T for e in range(E)], axis=1))  # [128,512]
    feb = np.concatenate([fe_b, fe_b]).reshape(128, 1).astype(f)
    sbias = np.concatenate([s_b[0], s_b[1]]).reshape(128, 1).astype(f)
    ebt = np.concatenate([e_b.T, e_b.T], axis=0).astype(f)   # [128, E]
    cs = 0.5 * np.concatenate([I64, I64], axis=0)
    ist = np.concatenate([I64, I64], axis=0)

    pm = np.arange(128) % 16
    jj = np.arange(8)
    cidx = (pm[:, None] + 16 * jj[None, :] - 64 * (jj[None, :] >= 4)).astype(f)

    ssel = np.zeros((4, 4 * 128), dtype=f)
    for b in range(4):
        ssel[b, b * 128:(b + 1) * 128] = 1.0

    s1 = (bn1_g / np.sqrt(1.0 + BN_EPS)).astype(f)
    l1 = ((g1_w * s1[:, None]) / float(HWT)).T.astype(f)     # [64, 128]
    b1 = (g1_b * s1 + bn1_b).reshape(128, 1).astype(f)
    ca1 = ca1_w.T.astype(f)                                  # [128, 8]
    bca1 = ca1_b.reshape(8, 1).astype(f)
    ca2 = ca2_w.T.astype(f)                                  # [8, 128]
    bca2 = ca2_b.reshape(128, 1).astype(f)
    s2 = (bn2_g / np.sqrt(1.0 + BN_EPS)).astype(f)
    l2 = (g2_w * s2[:, None]).T.astype(f)                    # [128, 64]
    b2 = (g2_b * s2 + bn2_b).reshape(64, 1).astype(f)
    g3b = np.concatenate([g3_w.T, g3_b.reshape(1, 8)], axis=0).astype(f)

    return dict(wf=wf, ws=ws, we=we, feb=feb, sbias=sbias, ebt=ebt, cs=cs,
                ist=ist, cidx=cidx, ssel=ssel, l1=l1, b1=b1, ca1=ca1,
                bca1=bca1, ca2=ca2, bca2=bca2, l2=l2, b2=b2, g3b=g3b)


def _timed_exec(nc, in_maps, n_cores, ncalls=24):
    """Run nc's NEFF on all cores via PJRT with pre-uploaded inputs and no
    host materialization; return min wall seconds per call."""
    import time
    import jax
    from jax.sharding import Mesh, PartitionSpec, NamedSharding
    from jax.experimental.shard_map import shard_map
    import concourse.mybir as mybir
    from concourse.bass2jax import (
        _bass_exec_p, install_neuronx_cc_hook, partition_id_tensor)

    install_neuronx_cc_hook()
    partition_name = (nc.partition_id_tensor.name
                      if nc.partition_id_tensor else None)
    in_names, out_names, out_avals, zero_outs = [], [], [], []
    for alloc in nc.m.functions[0].allocations:
        if not isinstance(alloc, mybir.MemoryLocationSet):
            continue
        name = alloc.memorylocations[0].name
        if alloc.kind == "ExternalInput":
            if name != partition_name:
                in_names.append(name)
        elif alloc.kind == "ExternalOutput":
            out_names.append(name)
            shape = tuple(alloc.tensor_shape)
            dtype = mybir.dt.np(alloc.dtype)
            out_avals.append(jax.core.ShapedArray(shape, dtype))
            zero_outs.append(np.zeros(shape, dtype))
    n_params = len(in_names)
    all_in = in_names + out_names + ([partition_name] if partition_name else [])

    def _body(*args):
        operands = list(args)
        if partition_name is not None:
            operands.append(partition_id_tensor())
        return tuple(_bass_exec_p.bind(
            *operands,
            out_avals=tuple(out_avals),
            in_names=tuple(all_in),
            out_names=tuple(out_names),
            lowering_input_output_aliases=(),
            sim_require_finite=True,
            sim_require_nnan=True,
            nc=nc,
        ))

    devices = jax.devices()[:n_cores]
    mesh = Mesh(np.array(devices), ("core",))
    nin = n_params + len(out_names)
    sharded = jax.jit(
        shard_map(_body, mesh=mesh, in_specs=(PartitionSpec("core"),) * nin,
                  out_specs=(PartitionSpec("core"),) * len(out_names),
                  check_rep=False),
        keep_unused=True)
    concat = [np.concatenate([np.asarray(in_maps[c][nm])
                              for c in range(n_cores)], axis=0)
              for nm in in_names]
    concat += [np.zeros((n_cores * z.shape[0], *z.shape[1:]), z.dtype)
               for z in zero_outs]
    sh = NamedSharding(mesh, PartitionSpec("core"))
    dev_in = [jax.device_put(a, sh) for a in concat]
    outs = sharded(*dev_in)
    jax.block_until_ready(outs)  # compile + warm
    best = None
    for _ in range(ncalls):
        t0 = time.perf_counter()
        outs = sharded(*dev_in)
        jax.block_until_ready(outs)
        dt = time.perf_counter() - t0
        best = dt if best is None else min(best, dt)
    return best


def kernel(**inputs):
    global LAST_RESULTS
    import sys
    if "/opt/trn_rl_repo" not in sys.path:
        sys.path.insert(0, "/opt/trn_rl_repo")
    from concourse import bass_utils

    import ml_dtypes
    bf = ml_dtypes.bfloat16
    x = np.ascontiguousarray(np.asarray(inputs["x"], dtype=np.float32).astype(bf))
    consts = _host_consts(**{k: np.asarray(v, np.float32)
                             for k, v in inputs.items() if k != "x"})
    packb = np.zeros((128, BCOLS), dtype=bf)
    off = 0
    for nm, (r, c) in PACKB:
        packb[0:r, off:off + c] = consts[nm].astype(bf)
        off += c
    packf = np.zeros((128, FCOLS), dtype=np.float32)
    off = 0
    for nm, (r, c) in PACKF:
        packf[0:r, off:off + c] = consts[nm]
        off += c
    consts = {"packb": packb, "packf": packf,
              "we": consts["we"].astype(bf), "ebt": consts["ebt"]}

    ebz = bool(np.all(np.asarray(inputs["e_b"]) == 0.0))
    key = ("nc", ebz)
    if key not in _CACHE:
        _CACHE[key] = _build_program(1, ebz)
    nc = _CACHE[key]

    xr = x.reshape(B, CIN, HWT)
    in_maps = []
    for c in range(NCORES):
        xc = xr[c * BPC:(c + 1) * BPC]                     # [4, 64, HWT]
        m = {"x": np.ascontiguousarray(xc.reshape(NPAIR, 2 * CIN, HWT))}
        m.update(consts)
        in_maps.append(m)

    res = bass_utils.run_bass_kernel_spmd(
        nc, in_maps, core_ids=list(range(NCORES)), trace=False)
    out = np.concatenate(
        [np.asarray(r["y"], dtype=np.float32).reshape(BPC, OUT, HWT)
         for r in res.results], axis=0)

    iters = int(os.environ.get("MOE_KERNEL_TIME_ITERS", "0"))
    ref_iters = int(os.environ.get("MOE_KERNEL_TIME_REF", "1"))
    exec_ns = None
    if iters > 1:
        keyk = ("nc", ebz, iters)
        if keyk not in _CACHE:
            _CACHE[keyk] = _build_program(iters, ebz)
        if ref_iters > 1:
            keyr = ("nc", ebz, ref_iters)
            if keyr not in _CACHE:
                _CACHE[keyr] = _build_program(ref_iters, ebz)
            t1 = _timed_exec(_CACHE[keyr], in_maps, NCORES)
        else:
            t1 = _timed_exec(nc, in_maps, NCORES)
        tk = _timed_exec(_CACHE[keyk], in_maps, NCORES)
        exec_ns = (tk - t1) / (iters - ref_iters) * 1e9
        print(f"[timing] wall {ref_iters}-iter {t1*1e3:.2f} ms, {iters}-iter "
              f"{tk*1e3:.2f} ms -> per-iter {exec_ns:.0f} ns")

    import types
    LAST_RESULTS = types.SimpleNamespace(
        results=res.results, exec_time_ns=exec_ns,
        mean_exec_time_ns=None, max_exec_time_core_id=None)
    return out.reshape(B, OUT, H, W)

